# revision 24
# baseline (speedup 1.0000x reference)
"""Trainium2 Bass kernel for nn_EncoderLayer_69965017252062.

Encoder layer: LN -> MHA -> residual -> LN -> LookupFFN (hash top-8 table
lookup) -> residual.

Sharding: data-parallel over the 4096 tokens across 8 cores. Core c handles
batch b = c//4, query rows [q0, q0+512) with q0 = (c%4)*512. Each core
computes K/V for its batch's full 2048 keys (replicated within the
4-core group), attention + FFN for its own 512 queries. No collectives.

LookupFFN trick: instead of gathering top-8 table rows per (token, table),
compute the full 256 bucket logits, take the hardware top-8 (nc.vector.max),
build softmax weights over the whole 256-wide row masked to entries >= the
8th-largest value, and contract the [tok, 256] weight matrix against the
table with the PE. The dense matmul does the "gather".
"""

import sys
import numpy as np

sys.path.insert(0, "/opt/trn_rl_repo")

H = 1024
NH = 16
DH = 64
T = 64          # num tables
TS = 256        # table size
BITS = 8
B, S = 2, 2048
EPS = 1e-12
N_CORES = 8
SQ = 512        # queries per core
SK = 2048       # keys per core
P = 128

_CACHE = {}


def _build_program():
    import concourse.bass as bass
    import concourse.bacc as bacc
    import concourse.mybir as mybir
    from concourse.tile import TileContext
    from concourse.masks import make_identity

    fp32 = mybir.dt.float32
    bf16 = mybir.dt.bfloat16
    AF = mybir.ActivationFunctionType
    ALU = mybir.AluOpType

    nc = bacc.Bacc(None, target_bir_lowering=False)

    hid_kv = nc.declare_dram_parameter("hid_kv", [SK, H], fp32, isOutput=False)
    hid_q = nc.declare_dram_parameter("hid_q", [SQ, H], fp32, isOutput=False)
    maskp = nc.declare_dram_parameter("maskp", [SK], fp32, isOutput=False)
    ln1g = nc.declare_dram_parameter("ln1g", [H], fp32, isOutput=False)
    ln1b = nc.declare_dram_parameter("ln1b", [H], fp32, isOutput=False)
    ln2g = nc.declare_dram_parameter("ln2g", [H], fp32, isOutput=False)
    ln2b = nc.declare_dram_parameter("ln2b", [H], fp32, isOutput=False)
    Wq = nc.declare_dram_parameter("Wq", [H, H], fp32, isOutput=False)
    Wk = nc.declare_dram_parameter("Wk", [H, H], fp32, isOutput=False)
    Wv = nc.declare_dram_parameter("Wv", [H, H], fp32, isOutput=False)
    Wo = nc.declare_dram_parameter("Wo", [H, H], fp32, isOutput=False)
    bq = nc.declare_dram_parameter("bq", [H], fp32, isOutput=False)
    bk = nc.declare_dram_parameter("bk", [H], fp32, isOutput=False)
    bv = nc.declare_dram_parameter("bv", [H], fp32, isOutput=False)
    bo = nc.declare_dram_parameter("bo", [H], fp32, isOutput=False)
    # Wh2: [H, 16, 128] permuted/padded hash weights (see host code)
    Wh2 = nc.declare_dram_parameter("Wh2", [H, 16, P], fp32, isOutput=False)
    bh2 = nc.declare_dram_parameter("bh2", [16, P], fp32, isOutput=False)
    signsT = nc.declare_dram_parameter("signsT", [P, 4 * TS], fp32,
                                       isOutput=False)
    tables = nc.declare_dram_parameter("tables", [T, TS, H], bf16,
                                       isOutput=False)
    tbias = nc.declare_dram_parameter("tbias", [H], fp32, isOutput=False)
    out_d = nc.declare_dram_parameter("out", [SQ, H], fp32, isOutput=True)

    lnT_d = nc.dram_tensor("lnT_scratch", [H, SK], fp32)
    kT_d = nc.dram_tensor("kT_scratch", [H, SK], fp32)
    qT_d = nc.dram_tensor("qT_scratch", [H, SQ], fp32)
    row_d = nc.dram_tensor("row_scratch", [NH, SQ], fp32)

    def bcast_ap(src_ap, parts):
        return bass.AP(tensor=src_ap.tensor, offset=src_ap.offset,
                       ap=[[0, parts]] + list(src_ap.ap))

    ones_row = None

    def load_bcast(pool, dram_vec, width, tag, pspool, pstag,
                   parts=P, bufs=None, rpool=None):
        # broadcast a [width] dram vector across partitions via a K=1
        # ones-matmul (no custom gpsimd ops)
        rowt = (rpool or pool).tile([1, width], fp32, tag="bcrow", bufs=2,
                                    name=tag + "_row")
        nc.gpsimd.dma_start(out=rowt, in_=dram_vec)
        bct = pool.tile([parts, width], fp32, tag=tag, bufs=bufs,
                        name=tag)
        for n0 in range(0, width, 512):
            w = min(512, width - n0)
            ps = pspool.tile([parts, w], fp32, tag=pstag,
                             name=tag + f"_ps{n0}")
            nc.tensor.matmul(ps, lhsT=ones_row[0:1, 0:parts],
                             rhs=rowt[0:1, n0:n0 + w], start=True, stop=True)
            nc.any.tensor_copy(bct[:, n0:n0 + w], ps)
        return bct

    with TileContext(nc) as tc:
        consts = tc.alloc_tile_pool(name="consts", bufs=1)
        pLate = tc.alloc_tile_pool(name="late", bufs=1)

        ident = consts.tile([P, P], fp32, tag="ident")
        make_identity(nc, ident)
        ones_row = consts.tile([1, P], fp32, tag="ones_row")
        nc.vector.memset(ones_row, 1.0)
        mb = consts.tile([P, SK // P], fp32, tag="mb")
        nc.gpsimd.dma_start(out=mb, in_=maskp[:].rearrange("(a p) -> p a", p=P))
        nc.vector.tensor_scalar(mb, mb, 1000.0, -1000.0,
                                op0=ALU.mult, op1=ALU.add)
        bh2t = consts.tile([P, 16], fp32, tag="bh2t")
        nc.gpsimd.dma_start(out=bh2t, in_=bh2[:, :].rearrange("g p -> p g"))
        eps_t = consts.tile([P, 1], fp32, tag="eps")
        nc.vector.memset(eps_t, EPS)
        bva = consts.tile([P, NH, 65], fp32, tag="bva")
        nc.vector.memset(bva, 1.0)

        # ============ Phases 1-4: LN1, K^T, V_aug, Q^T ============
        pV = tc.alloc_tile_pool(name="pV", bufs=1)
        pA = tc.alloc_tile_pool(name="pA", bufs=1)
        pAw = tc.alloc_tile_pool(name="pAw", bufs=3)
        psT = tc.alloc_tile_pool(name="psT", bufs=2, space="PSUM")
        psM = tc.alloc_tile_pool(name="psM", bufs=2, space="PSUM")

        g1 = load_bcast(pA, ln1g[:], H, "g_bc", psM, "mm")
        b1 = load_bcast(pA, ln1b[:], H, "b_bc", psM, "mm")
        tb_bc = load_bcast(consts, tbias[:], H, "tb_bc", psM, "mm",
                           rpool=pA)
        bv_bc = load_bcast(pA, bv[:], H, "bv_bc", psM, "mm")
        nc.vector.tensor_copy(
            bva[:, :, 0:DH],
            bv_bc[:, :].rearrange("p (h c) -> p h c", c=DH))
        bq8 = pA.tile([P, 8], fp32, tag="bq8")
        bk8 = pA.tile([P, 8], fp32, tag="bk8")
        nc.gpsimd.dma_start(out=bq8, in_=bq[:].rearrange("(a p) -> p a", p=P))
        nc.gpsimd.dma_start(out=bk8, in_=bk[:].rearrange("(a p) -> p a", p=P))
        bq8s = pA.tile([P, 8], fp32, tag="bq8s")
        nc.vector.tensor_scalar_mul(bq8s, bq8, 0.125)

        def layernorm_tile(pool, x_tile, g_bc, b_bc, out_tile):
            st = pool.tile([P, 2, 6], fp32, tag="bnstats")
            nc.vector.bn_stats(out=st[:, 0, :], in_=x_tile[:, 0:512])
            nc.vector.bn_stats(out=st[:, 1, :], in_=x_tile[:, 512:1024])
            mv = pool.tile([P, 2], fp32, tag="bnaggr")
            nc.vector.bn_aggr(out=mv, in_=st)
            std = pool.tile([P, 1], fp32, tag="std")
            nc.scalar.activation(out=std, in_=mv[:, 1:2], func=AF.Sqrt,
                                 bias=eps_t[:, 0:1])
            rstd = pool.tile([P, 1], fp32, tag="rstd")
            nc.vector.reciprocal(rstd, std)
            nc.vector.tensor_scalar(out_tile, x_tile, mv[:, 0:1], rstd,
                                    op0=ALU.subtract, op1=ALU.mult)
            nc.vector.tensor_mul(out_tile, out_tile, g_bc)
            nc.vector.tensor_add(out_tile, out_tile, b_bc)

        # Phase 1: LN1(hid_kv) -> lnT_d [H, SK]
        for i in range(SK // P):
            x = pAw.tile([P, H], fp32, tag="x", bufs=2)
            nc.gpsimd.dma_start(out=x, in_=hid_kv[i * P:(i + 1) * P, :])
            lnx = pAw.tile([P, H], fp32, tag="lnx", bufs=2)
            layernorm_tile(pAw, x, g1, b1, lnx)
            for j in range(8):
                pt = psT.tile([P, P], fp32, tag="tpose")
                nc.tensor.transpose(pt, lnx[:, j * P:(j + 1) * P], ident)
                st = pAw.tile([P, P], fp32, tag="tout", bufs=2)
                nc.any.tensor_copy(st, pt)
                nc.gpsimd.dma_start(
                    out=lnT_d[j * P:(j + 1) * P, i * P:(i + 1) * P], in_=st)

        # Phase 2: K^T = (ln @ Wk + bk)^T -> kT_d
        wk_t = []
        for j in range(8):
            wt = pA.tile([P, NH * 65], fp32, tag=f"w{j}")
            nc.gpsimd.dma_start(out=wt[:, 0:H], in_=Wk[j * P:(j + 1) * P, :])
            wk_t.append(wt)
        for st_i in range(4):
            lts = []
            for j in range(8):
                lt = pA.tile([P, 512], fp32, tag=f"lnt{j}")
                nc.gpsimd.dma_start(
                    out=lt, in_=lnT_d[j * P:(j + 1) * P,
                                      st_i * 512:(st_i + 1) * 512])
                lts.append(lt)
            for d in range(8):
                ps = psM.tile([P, 512], fp32, tag="mm")
                for j in range(8):
                    nc.tensor.matmul(ps, lhsT=wk_t[j][:, d * P:(d + 1) * P],
                                     rhs=lts[j], start=(j == 0), stop=(j == 7))
                ot = pAw.tile([P, 512], fp32, tag="stg", bufs=2)
                nc.scalar.activation(out=ot, in_=ps, func=AF.Identity,
                                     bias=bk8[:, d:d + 1])
                nc.gpsimd.dma_start(
                    out=kT_d[d * P:(d + 1) * P, st_i * 512:(st_i + 1) * 512],
                    in_=ot)

        # Phase 3: V_aug [tok, 16*65] resident (ones col per head at 64)
        wv_t = []
        for j in range(8):
            wt = pA.tile([P, NH * 65], fp32, tag=f"w{j}")
            nc.vector.memset(
                wt[:, :].rearrange("p (h c) -> p h c", c=65)[:, :, 64:65], 0.0)
            nc.gpsimd.dma_start(
                out=wt[:, :].rearrange("p (h c) -> p h c", c=65)[:, :, 0:DH],
                in_=Wv[j * P:(j + 1) * P, :].rearrange("p (h c) -> p h c",
                                                       c=DH))
            wv_t.append(wt)
        vaug = []
        for i in range(SK // P):
            vaug.append(pV.tile([P, NH * 65], fp32, tag=f"vaug{i}", name=f"vaug{i}"))
        bva_f = bva[:, :, :].rearrange("p h c -> p (h c)")
        for st_i in range(4):
            lts = []
            for j in range(8):
                lt = pA.tile([P, 512], fp32, tag=f"lnt{j}")
                nc.gpsimd.dma_start(
                    out=lt, in_=lnT_d[j * P:(j + 1) * P,
                                      st_i * 512:(st_i + 1) * 512])
                lts.append(lt)
            for m in range(4):
                i = st_i * 4 + m
                for r in range(4):
                    ps = psM.tile([P, 260], fp32, tag="mm")
                    for j in range(8):
                        nc.tensor.matmul(
                            ps, lhsT=lts[j][:, m * P:(m + 1) * P],
                            rhs=wv_t[j][:, r * 260:(r + 1) * 260],
                            start=(j == 0), stop=(j == 7))
                    nc.any.tensor_add(vaug[i][:, r * 260:(r + 1) * 260],
                                      ps, bva_f[:, r * 260:(r + 1) * 260])

        # Phase 4: Q^T = ((LN(hid_q) @ Wq + bq)/8)^T -> qT_d
        wq_t = []
        for j in range(8):
            wt = pA.tile([P, NH * 65], fp32, tag=f"w{j}")
            nc.gpsimd.dma_start(out=wt[:, 0:H], in_=Wq[j * P:(j + 1) * P, :])
            wq_t.append(wt)
        lnqT = [pA.tile([P, 512], fp32, tag=f"lnt{j}", name=f"lnqT{j}") for j in range(8)]
        for m in range(4):
            x = pAw.tile([P, H], fp32, tag="x", bufs=2)
            nc.gpsimd.dma_start(out=x, in_=hid_q[m * P:(m + 1) * P, :])
            lnx = pAw.tile([P, H], fp32, tag="lnx", bufs=2)
            layernorm_tile(pAw, x, g1, b1, lnx)
            for j in range(8):
                pt = psT.tile([P, P], fp32, tag="tpose")
                nc.tensor.transpose(pt, lnx[:, j * P:(j + 1) * P], ident)
                nc.any.tensor_copy(lnqT[j][:, m * P:(m + 1) * P], pt)
        for d in range(8):
            ps = psM.tile([P, 512], fp32, tag="mm")
            for j in range(8):
                nc.tensor.matmul(ps, lhsT=wq_t[j][:, d * P:(d + 1) * P],
                                 rhs=lnqT[j], start=(j == 0), stop=(j == 7))
            ot = pAw.tile([P, 512], fp32, tag="stg", bufs=2)
            nc.scalar.activation(out=ot, in_=ps, func=AF.Identity,
                                 scale=0.125, bias=bq8s[:, d:d + 1])
            nc.gpsimd.dma_start(out=qT_d[d * P:(d + 1) * P, :], in_=ot)

        psM.release()
        psT.release()
        pAw.release()
        pA.release()

        # ============ Phase 5: attention per head -> ctxT ============
        pAtt = tc.alloc_tile_pool(name="pAtt", bufs=1)
        pAttw = tc.alloc_tile_pool(name="pAttw", bufs=3)
        psS = tc.alloc_tile_pool(name="psS", bufs=3, space="PSUM")
        psCtx = tc.alloc_tile_pool(name="psCtx", bufs=2, space="PSUM")

        ctxT = [pAtt.tile([DH, SQ], fp32, tag=f"ctxT{h}", name=f"ctxT{h}") for h in range(NH)]
        for h in range(NH):
            kth = pAttw.tile([DH, SK], fp32, tag="kth", bufs=2)
            nc.gpsimd.dma_start(out=kth, in_=kT_d[h * DH:(h + 1) * DH, :])
            qth = pAttw.tile([DH, SQ], fp32, tag="qth", bufs=2)
            nc.gpsimd.dma_start(out=qth, in_=qT_d[h * DH:(h + 1) * DH, :])
            pctx = psCtx.tile([DH + 1, SQ], fp32, tag="pctx")
            for t in range(SK // P):
                pst = psS.tile([P, SQ], fp32, tag="pscore")
                nc.tensor.matmul(pst, lhsT=kth[:, t * P:(t + 1) * P],
                                 rhs=qth, start=True, stop=True)
                et = pAttw.tile([P, SQ], fp32, tag="et")
                nc.scalar.activation(out=et, in_=pst, func=AF.Exp,
                                     bias=mb[:, t:t + 1])
                nc.tensor.matmul(pctx, lhsT=vaug[t][:, h * 65:(h + 1) * 65],
                                 rhs=et, start=(t == 0),
                                 stop=(t == SK // P - 1))
            row = pAttw.tile([DH + 1, SQ], fp32, tag="row", bufs=2)
            nc.any.tensor_copy(row[DH:DH + 1, :], pctx[DH:DH + 1, :])
            nc.gpsimd.dma_start(out=row_d[h, :], in_=row[DH:DH + 1, :])
            row0 = pAttw.tile([1, SQ], fp32, tag="row0", bufs=1)
            nc.gpsimd.dma_start(out=row0, in_=row_d[h, :])
            rbp = psS.tile([DH, SQ], fp32, tag="rbb", name="rbp")
            nc.tensor.matmul(rbp, lhsT=ones_row[0:1, 0:DH],
                             rhs=row0[0:1, :], start=True, stop=True)
            rb = pAttw.tile([DH, SQ], fp32, tag="rbc", bufs=2)
            nc.vector.reciprocal(rb, rbp)
            nc.vector.tensor_mul(ctxT[h], pctx[0:DH, :], rb)

        psCtx.release()
        psS.release()

        # ============ Phase 6: attn_out = ctx @ Wo + bo + hid_q ============
        psTmp = tc.alloc_tile_pool(name="psTmp", bufs=2, space="PSUM")
        bo_bc = load_bcast(pAttw, bo[:], H, "bo_bc", psTmp, "bops", bufs=1,
                           rpool=pLate)
        psTmp.release()
        psO = tc.alloc_tile_pool(name="psO", bufs=1, space="PSUM")
        pouts = [psO.tile([P, 512], fp32, tag=f"po{mn}", name=f"po{mn}") for mn in range(8)]
        for h in range(NH):
            woh = pAttw.tile([DH, H], fp32, tag="woh", bufs=2)
            nc.gpsimd.dma_start(out=woh, in_=Wo[h * DH:(h + 1) * DH, :])
            for m in range(4):
                for n in range(2):
                    nc.tensor.matmul(
                        pouts[m * 2 + n],
                        lhsT=ctxT[h][:, m * P:(m + 1) * P],
                        rhs=woh[:, n * 512:(n + 1) * 512],
                        start=(h == 0), stop=(h == NH - 1))
        attn = []
        for m in range(4):
            at = pLate.tile([P, H], fp32, tag=f"attn{m}")
            hq = pAttw.tile([P, H], fp32, tag="hq", bufs=2)
            nc.gpsimd.dma_start(out=hq, in_=hid_q[m * P:(m + 1) * P, :])
            for n in range(2):
                nc.vector.tensor_add(at[:, n * 512:(n + 1) * 512],
                                     pouts[m * 2 + n],
                                     bo_bc[:, n * 512:(n + 1) * 512])
            nc.vector.tensor_add(at, at, hq)
            attn.append(at)

        psO.release()
        pAttw.release()
        pAtt.release()
        pV.release()

        # ============ Phase 7: LN2 -> yT; hash -> hsT2 ============
        pH = tc.alloc_tile_pool(name="pH", bufs=1)
        pHw = tc.alloc_tile_pool(name="pHw", bufs=3)
        psT2 = tc.alloc_tile_pool(name="psT2", bufs=2, space="PSUM")
        psM2 = tc.alloc_tile_pool(name="psM2", bufs=2, space="PSUM")

        g2 = load_bcast(pHw, ln2g[:], H, "g_bc", psM2, "hsp", bufs=1)
        b2 = load_bcast(pHw, ln2b[:], H, "b_bc", psM2, "hsp", bufs=1)
        yT = [pH.tile([P, 512], fp32, tag=f"yt{j}", name=f"yT{j}") for j in range(8)]
        for m in range(4):
            y = pHw.tile([P, H], fp32, tag="lnx")
            layernorm_tile(pHw, attn[m], g2, b2, y)
            for j in range(8):
                pt = psT2.tile([P, P], fp32, tag="tpose")
                nc.tensor.transpose(pt, y[:, j * P:(j + 1) * P], ident)
                nc.any.tensor_copy(yT[j][:, m * P:(m + 1) * P], pt)
        sgn = pH.tile([P, 4 * TS], fp32, tag="sgn")
        nc.gpsimd.dma_start(out=sgn, in_=signsT[:, :])
        hsT2 = pH.tile([P, 16, SQ], fp32, tag="hst2")
        for g in range(16):
            wh_t = []
            for j in range(8):
                wt = pHw.tile([P, P], fp32, tag=f"whcol{j}")
                nc.gpsimd.dma_start(out=wt, in_=Wh2[j * P:(j + 1) * P, g, :])
                wh_t.append(wt)
            ps = psM2.tile([P, SQ], fp32, tag="hsp")
            for j in range(8):
                nc.tensor.matmul(ps, lhsT=wh_t[j], rhs=yT[j],
                                 start=(j == 0), stop=(j == 7))
            nc.scalar.activation(out=hsT2[:, g, :], in_=ps,
                                 func=AF.Identity, bias=bh2t[:, g:g + 1])

        psM2.release()
        psT2.release()
        pHw.release()

        # ===== Phase 8: logits -> top8 mask softmax -> combine =====
        pT = tc.alloc_tile_pool(name="pT", bufs=9)
        pTw = tc.alloc_tile_pool(name="pTw", bufs=3)
        pTs = tc.alloc_tile_pool(name="pTs", bufs=4)
        psL = tc.alloc_tile_pool(name="psL", bufs=2, space="PSUM")
        psW = tc.alloc_tile_pool(name="psW", bufs=2, space="PSUM")
        psF = tc.alloc_tile_pool(name="psF", bufs=1, space="PSUM")

        ffn = [pLate.tile([P, H], fp32, tag=f"ffn{i}", name=f"ffn{i}")
               for i in range(4)]
        for gp in range(8):          # 8 tables per pass
            tab_t = []
            for tt in range(8):
                t = gp * 8 + tt
                tbt = pT.tile([P, 2, H], bf16, tag="tab")
                nc.gpsimd.dma_start(
                    out=tbt,
                    in_=tables[t, :, :].rearrange("(c p) h -> p c h", p=P))
                tab_t.append(tbt)
            for i in range(4):
                pf = psF.tile([P, H], fp32, tag="pffn")
                for sub in range(2):     # 4-table logit groups
                    gq = gp * 2 + sub
                    pl4 = psL.tile([P, 4 * TS], fp32, tag="plog")
                    for n in range(2):
                        nc.tensor.matmul(
                            pl4[:, n * 512:(n + 1) * 512],
                            lhsT=hsT2[:, gq, i * P:(i + 1) * P],
                            rhs=sgn[:, n * 512:(n + 1) * 512],
                            start=True, stop=True)
                    ew = pTw.tile([P, 4 * TS], fp32, tag="ew")
                    for s in range(4):
                        pl = pl4[:, s * TS:(s + 1) * TS]
                        top8 = pTs.tile([P, 8], fp32, tag="top8")
                        nc.vector.max(out=top8, in_=pl)
                        e8 = pTs.tile([P, 8], fp32, tag="e8")
                        ssum = pTs.tile([P, 1], fp32, tag="ssum")
                        nc.scalar.activation(out=e8, in_=top8, func=AF.Exp,
                                             accum_out=ssum)
                        lnr = pTs.tile([P, 1], fp32, tag="lnr")
                        nc.vector.reciprocal(lnr, ssum)
                        nc.scalar.activation(out=lnr, in_=lnr, func=AF.Ln)
                        ews = ew[:, s * TS:(s + 1) * TS]
                        nc.scalar.activation(out=ews, in_=pl, func=AF.Exp,
                                             bias=lnr[:, 0:1])
                        m01 = pTw.tile([P, TS], fp32, tag="m01")
                        nc.vector.tensor_scalar(m01, pl, top8[:, 7:8], None,
                                                op0=ALU.is_ge)
                        nc.gpsimd.tensor_mul(ews, ews, m01)
                    wT = pTw.tile([P, 8, P], bf16, tag="wT")
                    for cc in range(8):
                        pt = psW.tile([P, P], fp32, tag="wtp")
                        nc.tensor.transpose(pt, ew[:, cc * P:(cc + 1) * P],
                                            ident)
                        nc.any.tensor_copy(wT[:, cc, :], pt)
                    for s in range(4):
                        for cc in range(2):
                            for n in range(2):
                                nc.tensor.matmul(
                                    pf[:, n * 512:(n + 1) * 512],
                                    lhsT=wT[:, s * 2 + cc, :],
                                    rhs=tab_t[sub * 4 + s][:, cc,
                                                           n * 512:(n + 1) * 512],
                                    start=(sub == 0 and s == 0 and cc == 0),
                                    stop=(sub == 1 and s == 3 and cc == 1))
                if gp == 0:
                    nc.any.tensor_copy(ffn[i], pf)
                else:
                    nc.any.tensor_add(ffn[i], ffn[i], pf)

        psF.release()
        psW.release()
        psL.release()
        pTs.release()
        pTw.release()
        pT.release()
        pH.release()

        # ============ Phase 9: out = ffn + tbias + attn ============
        for i in range(4):
            nc.vector.tensor_add(ffn[i], ffn[i], tb_bc)
            nc.vector.tensor_add(ffn[i], ffn[i], attn[i])
            nc.gpsimd.dma_start(out=out_d[i * P:(i + 1) * P, :], in_=ffn[i])

        pLate.release()
        consts.release()

    nc.finalize()
    return nc


def _get_program():
    if "nc" not in _CACHE:
        _CACHE["nc"] = _build_program()
    return _CACHE["nc"]


def kernel(**inputs):
    import ml_dtypes

    hidden_states = np.asarray(inputs["hidden_states"], dtype=np.float32)
    attention_mask = np.asarray(inputs["attention_mask"], dtype=np.float32)
    Wh = np.asarray(inputs["Wh"], dtype=np.float32)
    bh = np.asarray(inputs["bh"], dtype=np.float32)

    # hash weights: column t*8+b -> block g=t//4, row (t%4)*32+b, zero pad
    Wh2 = np.zeros((H, 16, P), dtype=np.float32)
    bh2 = np.zeros((16, P), dtype=np.float32)
    for t in range(T):
        g, band = t // 4, (t % 4) * 32
        Wh2[:, g, band:band + BITS] = Wh[:, t * BITS:(t + 1) * BITS]
        bh2[g, band:band + BITS] = bh[t * BITS:(t + 1) * BITS]

    # signsT [128, 1024]: block-diagonal. Band s rows (s*32 .. s*32+7) carry
    # the sign codes only in column slot s (s*256 .. (s+1)*256); pad rows 0.
    signsT = np.zeros((P, 4 * TS), dtype=np.float32)
    codes = np.arange(TS)
    for b_ in range(BITS):
        row = (((codes >> b_) & 1) * 2 - 1).astype(np.float32)
        for band in range(4):
            signsT[band * 32 + b_, band * TS:(band + 1) * TS] = row

    tables_bf16 = np.asarray(inputs["tables_weight"],
                             dtype=np.float32).astype(ml_dtypes.bfloat16)

    shared = {
        "ln1g": np.asarray(inputs["ln1_g"], np.float32),
        "ln1b": np.asarray(inputs["ln1_b"], np.float32),
        "ln2g": np.asarray(inputs["ln2_g"], np.float32),
        "ln2b": np.asarray(inputs["ln2_b"], np.float32),
        "Wq": np.asarray(inputs["Wq"], np.float32),
        "Wk": np.asarray(inputs["Wk"], np.float32),
        "Wv": np.asarray(inputs["Wv"], np.float32),
        "Wo": np.asarray(inputs["Wo"], np.float32),
        "bq": np.asarray(inputs["bq"], np.float32),
        "bk": np.asarray(inputs["bk"], np.float32),
        "bv": np.asarray(inputs["bv"], np.float32),
        "bo": np.asarray(inputs["bo"], np.float32),
        "Wh2": Wh2, "bh2": bh2, "signsT": signsT,
        "tables": tables_bf16,
        "tbias": np.asarray(inputs["tables_bias"], np.float32),
    }

    in_maps = []
    for c in range(N_CORES):
        b = c // 4
        q0 = (c % 4) * SQ
        m = dict(shared)
        m["hid_kv"] = np.ascontiguousarray(hidden_states[b])
        m["hid_q"] = np.ascontiguousarray(hidden_states[b, q0:q0 + SQ])
        m["maskp"] = np.ascontiguousarray(attention_mask[b])
        in_maps.append(m)

    from concourse.bass_utils import run_bass_kernel_spmd
    nc = _get_program()
    res = run_bass_kernel_spmd(nc, in_maps, list(range(N_CORES)))

    out = np.zeros((B, S, H), dtype=np.float32)
    for c in range(N_CORES):
        b = c // 4
        q0 = (c % 4) * SQ
        out[b, q0:q0 + SQ] = np.asarray(res.results[c]["out"])
    return out
